# revision 30
# baseline (speedup 1.0000x reference)
"""Trainium2 Bass kernel for nn_CustomFullyConnectedLayerGoogleTopK.

Reference computation:
    a = clip(K * softmax(alpha), 0, 1)                    # (4096,)
    W[rows, cols] += (V * a[:, None])  with rows=(j+i)%N, cols=j
    out = x @ W.T                                          # (256, 4096)

The scatter indices form a bijection (for each col j, row (j+i)%N hits every
row exactly once as i varies), so there is no actual accumulation:

    W[r, c] = V[(r - c) % N, c] * a[(r - c) % N]
    out[b, r] = sum_c x[b, c] * V[(r-c)%N, c] * a[(r-c)%N]

Sharding: output columns r are sharded 8 ways (512 per core) -> no collective;
each core reads only the diagonal band of V it needs, all of x, and produces a
disjoint out[:, r0:r0+512] slice.

The kernel is memory-bound, so the band and xT inputs are fed to the device as
bfloat16 (host-side cast; measured end-to-end max rel err ~3e-3 vs the fp32
reference, inside the 2e-2 gate), halving HBM traffic to ~7.6 MB per core.
PSUM accumulation stays fp32 across the full 4096-deep contraction.

Device-side layout trick: with the contraction rows presented in REVERSED
order (c = N-1-p for SBUF partition-row p), the skewed scale field the band
tiles need becomes the ascending Toeplitz  scale[p, j] = a2[1 + p + j]  where
a2 is the rolled-by-r0, doubled raw alpha.  One compact strip
agf[128, 4480] = a2[1 + p + u]  serves every contraction block k via the
column-offset view agf[:, 128k : 128k+512].

Scale algebra:  clip(K*softmax(alpha), 0, 1) = cK * min(exp(alpha), invK)
with invK = sum(exp(alpha))/K and cK = K/sum(exp(alpha)).  So the pipeline is
  - ACT: in-place Exp over strip slices (no bias -> starts as soon as the
    strip DMA lands; no Ln table ever loads),
  - DVE: in-place min(strip, invK) as two big contiguous passes, then one
    tensor_tensor  wt_k = band_k * strip[:, 128k:128k+512]  PER BLOCK with
    fully contiguous 2D operands (a strided multi-block view of the strip
    knocks the DVE out of its fast perf mode: measured 2292 ns per 4-block
    batch vs ~contiguous per-block passes; GpSimd elementwise is 21x slower
    than DVE and also locks DVE out of the shared SBUF port -> never used),
  - the cK factor rides the PSUM evictions (DVE for psum0, ACT Copy-with-
    scale for psum1, in parallel).

The strip is materialized by the HOST in plain partition-major layout
(strip_t[p, u] = a2[1+p+u], a pure replication of the raw alpha values) and
DMA'd as an ordinary contiguous 1.12 MB read: reading it amplified from the
tiny 16 KB alpha region on-device measured only ~85 GB/s on both the SWDGE
and HWDGE paths (the whole read concentrates on 1-2 HBM pages), which put it
on the critical path.

DMA layout: Sync carries the 2 strip chunks interleaved with the 9 band
chunks in consumption order plus one output store; Scalar carries the
compact alpha + 4 xT chunks up front and the other output store.  Band/xT
are pre-tiled on the host to partition-major [128, k, :] so every DMA
descriptor is a >=2 KB contiguous run.  The last two band chunks are 2
blocks instead of 4 to shorten the post-last-byte tail, and outputs are
stored as bf16 (upcast on host) to halve the tail store.
"""

import math
import os
import sys

import numpy as np

for _p in ("/opt/trn_rl_repo", "/root/.axon_site/_ro/trn_rl_repo"):
    if os.path.isdir(_p) and _p not in sys.path:
        sys.path.append(_p)

import ml_dtypes

import concourse.bacc as bacc
import concourse.bass as bass
import concourse.mybir as mybir
import concourse.tile as tile
from concourse.bass_utils import run_bass_kernel_spmd

F32 = mybir.dt.float32
BF16 = mybir.dt.bfloat16
NP_BF16 = ml_dtypes.bfloat16

N = 4096          # IN_F == OUT_F == N_PERM == DIAG
B = 256           # batch
NCORES = 8
RW = N // NCORES  # 512 output columns per core
K_TOPK = 3687     # ceil(0.9 * 4096 * 4096 / 4096)
CB = 128          # contraction block (SBUF partition count)
NCB = N // CB     # 32 contraction blocks
WCOLS = (NCB - 1) * CB + RW  # 4480: width of the Toeplitz scale strip

# band chunk boundaries (in blocks): small first chunks (fast DVE start),
# bigger middle (fewer DMAs -> fewer HWDGE-sem recycles), small tail
BCHUNKS = [(0, 4), (4, 8), (8, 14), (14, 20), (20, 26), (26, 30), (30, 32)]
# strip chunk boundaries (in strip columns): small first chunk
WCHUNKS = [(0, 1024), (1024, 2240), (2240, 4480)]
# xT chunk boundaries (in blocks)
XCHUNKS = [(0, 8), (8, 20), (20, 32)]
# Sync carries the first band chunk right behind the small first strip chunk
# (both gate the first wt multiply), then the rest of the strip, then the
# band in consumption order, then one output store.  Scalar (=ACT) carries
# ONLY DMAs that issue before its compute starts (alpha + xT) plus the last
# output store: a DMA issue stalled on a recycled HWDGE semaphore blocks
# everything behind it in that engine's FIFO.
SYNC_ORDER = [("w", 0), ("b", 0), ("w", 1), ("w", 2), ("b", 1), ("b", 2),
              ("b", 3), ("b", 4), ("b", 5), ("b", 6)]
SCAL_ORDER = [("a", 0), ("x", 0), ("x", 1), ("x", 2)]
SLICE = 512       # strip exp/min processing slice width (9 slices)


def _strided_cols(ap2d, col_off, t_step, n_t, inner):
    """[128, W] SBUF tile -> [128, n_t, inner] view starting at col_off with
    column stride t_step between t-slices (overlap allowed)."""
    pstep = ap2d.ap[0][0]
    return bass.AP(
        ap2d.tensor, ap2d.offset + col_off,
        [[pstep, 128], [t_step, n_t], [1, inner]],
    )


def _build_program():
    nc = bacc.Bacc("TRN2", target_bir_lowering=False, debug=False)

    band_t = nc.dram_tensor("band_t", [128, NCB, RW], BF16, kind="ExternalInput").ap()
    xt_t = nc.dram_tensor("xt_t", [128, NCB, B], BF16, kind="ExternalInput").ap()
    al32 = nc.dram_tensor("al32", [N], F32, kind="ExternalInput").ap()
    strip_t = nc.dram_tensor("strip_t", [128, WCOLS], BF16, kind="ExternalInput").ap()
    out = nc.dram_tensor("out", [B, RW], BF16, kind="ExternalOutput").ap()

    with tile.TileContext(nc) as tc:
        with (
            tc.tile_pool(name="small", bufs=1) as sp,
            tc.tile_pool(name="agp", bufs=1) as agp,
            tc.tile_pool(name="bsb", bufs=1) as bsbp,
            tc.tile_pool(name="wt", bufs=8) as wtp,
            tc.tile_pool(name="xtp", bufs=1) as xtp,
            tc.tile_pool(name="opool", bufs=2) as op,
            tc.tile_pool(name="psum", bufs=1, space="PSUM") as pp,
            tc.tile_pool(name="psum_s", bufs=1, space="PSUM") as pps,
        ):
            # ---- all input DMAs issued first, consumption order per ring ----
            agf = agp.tile([128, WCOLS], BF16)
            bsb = bsbp.tile([128, NCB, RW], BF16)
            xt = xtp.tile([128, NCB, B], BF16)
            alpha_sb = sp.tile([128, N // 128], F32)

            def issue(eng, kind, i):
                if kind == "w":
                    c0, c1 = WCHUNKS[i]
                    eng.dma_start(agf[:, c0:c1], strip_t[:, c0:c1])
                elif kind == "b":
                    k0, k1 = BCHUNKS[i]
                    eng.dma_start(bsb[:, k0:k1, :], band_t[:, k0:k1, :])
                elif kind == "x":
                    k0, k1 = XCHUNKS[i]
                    eng.dma_start(xt[:, k0:k1, :], xt_t[:, k0:k1, :])
                else:
                    eng.dma_start(
                        alpha_sb[:], al32.rearrange("(p f) -> p f", p=128)
                    )

            for kind, i in SYNC_ORDER:
                issue(nc.sync, kind, i)
            for kind, i in SCAL_ORDER:
                issue(nc.scalar, kind, i)

            # ---- normalizer: invK = sum(exp(alpha))/K, cK = K/sum ----
            exp_sb = sp.tile([128, N // 128], F32)
            rowsum = sp.tile([128, 1], F32)
            # alpha is uniform in [0,1): no max-subtraction needed for stability
            nc.scalar.activation(
                exp_sb[:], alpha_sb[:], mybir.ActivationFunctionType.Exp,
                accum_out=rowsum[:],
            )
            ones = sp.tile([128, 128], F32)
            nc.vector.memset(ones[:], 1.0)
            tot_ps = pps.tile([128, 1], F32)
            # total = ones.T @ rowsum -> per-partition copy of the full sum
            nc.tensor.matmul(tot_ps[:], ones[:], rowsum[:], start=True, stop=True)
            invK = sp.tile([128, 1], F32)
            nc.vector.tensor_scalar_mul(invK[:], tot_ps[:], 1.0 / float(K_TOPK))
            inv_s = sp.tile([128, 1], F32)
            nc.vector.reciprocal(inv_s[:], tot_ps[:])
            cK = sp.tile([128, 1], F32)
            nc.vector.tensor_scalar_mul(cK[:], inv_s[:], float(K_TOPK))

            # ---- strip: slice-wise exp (ACT) then min(strip, invK) (DVE;
            # contiguous tensor_scalar at ~350ns vs 745ns for a fused
            # scalar_tensor_tensor, so separate passes win).  The two late
            # xT chunk issues ride between early slices. ----
            n_slices = (WCOLS + SLICE - 1) // SLICE
            for s in range(n_slices):
                cols = slice(s * SLICE, min((s + 1) * SLICE, WCOLS))
                nc.scalar.activation(
                    agf[:, cols], agf[:, cols],
                    mybir.ActivationFunctionType.Exp,
                )
                nc.vector.tensor_scalar_min(agf[:, cols], agf[:, cols],
                                            invK[:, 0:1])

            # ---- main loop: per-block wt = band_k * strip_k (2D-contiguous
            # operands keep DVE in its fast perf mode) -> matmul pair ----
            psum0 = pp.tile([128, RW], F32)
            psum1 = pp.tile([128, RW], F32)
            for k in range(NCB):
                wt = wtp.tile([128, RW], BF16, tag=f"wt{k % 8}")
                nc.vector.tensor_tensor(
                    wt[:], bsb[:, k, :], agf[:, k * CB:k * CB + RW],
                    mybir.AluOpType.mult,
                )
                nc.tensor.matmul(psum0[:], xt[:, k, 0:128], wt[:],
                                 start=(k == 0), stop=(k == NCB - 1))
                nc.tensor.matmul(psum1[:], xt[:, k, 128:256], wt[:],
                                 start=(k == 0), stop=(k == NCB - 1))

            # ---- PSUM -> (x cK) -> SBUF bf16 -> DRAM, one store per ring ----
            o0 = op.tile([128, RW], BF16)
            nc.vector.tensor_scalar_mul(o0[:], psum0[:], cK[:, 0:1])
            nc.sync.dma_start(out[0:128, :], o0[:])
            o1 = op.tile([128, RW], BF16)
            nc.scalar.activation(o1[:], psum1[:],
                                 mybir.ActivationFunctionType.Copy,
                                 bias=0.0, scale=cK[:, 0:1])
            nc.scalar.dma_start(out[128:256, :], o1[:])

    nc.compile()
    return nc


_NC_CACHE = []


def _get_program():
    if not _NC_CACHE:
        _NC_CACHE.append(_build_program())
    return _NC_CACHE[0]


def prepare_in_maps(x: np.ndarray, V: np.ndarray, alpha: np.ndarray):
    """Shard + lay out the full inputs into 8 per-core input maps (bf16)."""
    x = np.ascontiguousarray(np.asarray(x, dtype=np.float32))
    V = np.ascontiguousarray(np.asarray(V, dtype=np.float32))
    alpha = np.ascontiguousarray(np.asarray(alpha, dtype=np.float32))

    # rows presented in reversed order (c = N-1-p); see module docstring
    xT_rev = np.ascontiguousarray(x.T[::-1, :]).astype(NP_BF16)  # (N, B)
    xt_t = np.ascontiguousarray(
        xT_rev.reshape(NCB, 128, B).transpose(1, 0, 2)
    )  # [p, k, b]

    # VtD[c, t] = V[t % N, c] for t in [0, 2N): doubled transpose for wrap-free
    # band extraction. band_m[c, j] = V[(r0 + j - c) % N, c] = VtD[c, N+r0+j-c]
    Vt = np.ascontiguousarray(V.T).astype(NP_BF16)
    VtD = np.concatenate([Vt, Vt], axis=1)  # (N, 2N) bf16
    flat = VtD.reshape(-1)
    isz = flat.itemsize

    in_maps = []
    for m in range(NCORES):
        r0 = m * RW
        start = N + r0  # element offset of band_m[0, 0] in flat
        band_m = np.lib.stride_tricks.as_strided(
            flat[start:], shape=(N, RW), strides=((2 * N - 1) * isz, isz),
        )
        band_rev = np.ascontiguousarray(band_m[::-1, :])  # (N, RW) bf16
        band_t = np.ascontiguousarray(
            band_rev.reshape(NCB, 128, RW).transpose(1, 0, 2)
        )  # [p, k, j]
        am = np.roll(alpha, -r0)
        a2 = np.concatenate([am, am]).astype(NP_BF16)
        # strip_t[p, u] = a2[1 + p + u]  (raw values; exp/min run on device)
        strip = np.ascontiguousarray(np.lib.stride_tricks.as_strided(
            a2[1:], shape=(128, WCOLS), strides=(a2.itemsize, a2.itemsize),
        ))
        in_maps.append({
            "band_t": band_t,
            "xt_t": xt_t,
            "al32": alpha,
            "strip_t": strip,
        })
    return in_maps


def gather_output(results) -> np.ndarray:
    return np.concatenate(
        [results[m]["out"].astype(np.float32) for m in range(NCORES)], axis=1
    )


def kernel(x: np.ndarray, V: np.ndarray, alpha: np.ndarray) -> np.ndarray:
    in_maps = prepare_in_maps(x, V, alpha)
    nc = _get_program()
    res = run_bass_kernel_spmd(nc, in_maps, core_ids=list(range(NCORES)))
    return gather_output(res.results)


# revision 33
# speedup vs baseline: 1.0351x; 1.0351x over previous
"""Trainium2 Bass kernel for nn_CustomFullyConnectedLayerGoogleTopK.

Reference computation:
    a = clip(K * softmax(alpha), 0, 1)                    # (4096,)
    W[rows, cols] += (V * a[:, None])  with rows=(j+i)%N, cols=j
    out = x @ W.T                                          # (256, 4096)

The scatter indices form a bijection (for each col j, row (j+i)%N hits every
row exactly once as i varies), so there is no actual accumulation:

    W[r, c] = V[(r - c) % N, c] * a[(r - c) % N]
    out[b, r] = sum_c x[b, c] * V[(r-c)%N, c] * a[(r-c)%N]

Sharding: output columns r are sharded 8 ways (512 per core) -> no collective;
each core reads only the diagonal band of V it needs, all of x, and produces a
disjoint out[:, r0:r0+512] slice.

The kernel is memory-bound, so the band and xT inputs are fed to the device as
bfloat16 (host-side cast; measured end-to-end max rel err ~3e-3 vs the fp32
reference, inside the 2e-2 gate), halving HBM traffic to ~7.6 MB per core.
PSUM accumulation stays fp32 across the full 4096-deep contraction.

Device-side layout trick: with the contraction rows presented in REVERSED
order (c = N-1-p for SBUF partition-row p), the skewed scale field the band
tiles need becomes the ascending Toeplitz  scale[p, j] = a2[1 + p + j]  where
a2 is the rolled-by-r0, doubled raw alpha.  One compact strip
agf[128, 4480] = a2[1 + p + u]  serves every contraction block k via the
column-offset view agf[:, 128k : 128k+512].

Scale algebra:  clip(K*softmax(alpha), 0, 1) = cK * min(exp(alpha), invK)
with invK = sum(exp(alpha))/K and cK = K/sum(exp(alpha)).  So the pipeline is
  - ACT: in-place Exp over strip slices (no bias -> starts as soon as the
    strip DMA lands; no Ln table ever loads),
  - DVE: in-place min(strip, invK) as two big contiguous passes, then one
    tensor_tensor  wt_k = band_k * strip[:, 128k:128k+512]  PER BLOCK with
    fully contiguous 2D operands (a strided multi-block view of the strip
    knocks the DVE out of its fast perf mode: measured 2292 ns per 4-block
    batch vs ~contiguous per-block passes; GpSimd elementwise is 21x slower
    than DVE and also locks DVE out of the shared SBUF port -> never used),
  - the cK factor rides the PSUM evictions (DVE for psum0, ACT Copy-with-
    scale for psum1, in parallel).

The strip is materialized by the HOST in plain partition-major layout
(strip_t[p, u] = a2[1+p+u], a pure replication of the raw alpha values) and
DMA'd as an ordinary contiguous 1.12 MB read: reading it amplified from the
tiny 16 KB alpha region on-device measured only ~85 GB/s on both the SWDGE
and HWDGE paths (the whole read concentrates on 1-2 HBM pages), which put it
on the critical path.

DMA layout: Sync carries the 2 strip chunks interleaved with the 9 band
chunks in consumption order plus one output store; Scalar carries the
compact alpha + 4 xT chunks up front and the other output store.  Band/xT
are pre-tiled on the host to partition-major [128, k, :] so every DMA
descriptor is a >=2 KB contiguous run.  The last two band chunks are 2
blocks instead of 4 to shorten the post-last-byte tail, and outputs are
stored as bf16 (upcast on host) to halve the tail store.
"""

import math
import os
import sys

import numpy as np

for _p in ("/opt/trn_rl_repo", "/root/.axon_site/_ro/trn_rl_repo"):
    if os.path.isdir(_p) and _p not in sys.path:
        sys.path.append(_p)

import ml_dtypes

import concourse.bacc as bacc
import concourse.bass as bass
import concourse.mybir as mybir
import concourse.tile as tile
from concourse.bass_utils import run_bass_kernel_spmd

F32 = mybir.dt.float32
BF16 = mybir.dt.bfloat16
NP_BF16 = ml_dtypes.bfloat16

N = 4096          # IN_F == OUT_F == N_PERM == DIAG
B = 256           # batch
NCORES = 8
RW = N // NCORES  # 512 output columns per core
K_TOPK = 3687     # ceil(0.9 * 4096 * 4096 / 4096)
CB = 128          # contraction block (SBUF partition count)
NCB = N // CB     # 32 contraction blocks
WCOLS = (NCB - 1) * CB + RW  # 4480: width of the Toeplitz scale strip

# band chunk boundaries (in blocks): small first chunks (fast DVE start),
# bigger middle (fewer DMAs -> fewer HWDGE-sem recycles), small tail
BCHUNKS = [(0, 4), (4, 8), (8, 14), (14, 20), (20, 26), (26, 30), (30, 32)]
# strip chunk boundaries (in strip columns): small first chunk
WCHUNKS = [(0, 1024), (1024, 2240), (2240, 4480)]
# xT chunk boundaries (in blocks)
XCHUNKS = [(0, 8), (8, 20), (20, 32)]
# Sync carries the first band chunk right behind the small first strip chunk
# (both gate the first wt multiply), then the rest of the strip, then the
# band in consumption order, then one output store.  Scalar (=ACT) carries
# ONLY DMAs that issue before its compute starts (alpha + xT) plus the last
# output store: a DMA issue stalled on a recycled HWDGE semaphore blocks
# everything behind it in that engine's FIFO.  (Moving xT to the SWDGE ring
# as a third DMA path was tried and hard-crashed the device —
# NRT_EXEC_UNIT_UNRECOVERABLE — do not revisit.)
SYNC_ORDER = [("w", 0), ("b", 0), ("w", 1), ("w", 2), ("b", 1), ("b", 2),
              ("b", 3), ("b", 4), ("b", 5), ("b", 6)]
SCAL_ORDER = [("a", 0), ("x", 0), ("x", 1), ("x", 2)]
GP_ORDER = []
SLICE = 512       # strip exp/min processing slice width (9 slices)


def _strided_cols(ap2d, col_off, t_step, n_t, inner):
    """[128, W] SBUF tile -> [128, n_t, inner] view starting at col_off with
    column stride t_step between t-slices (overlap allowed)."""
    pstep = ap2d.ap[0][0]
    return bass.AP(
        ap2d.tensor, ap2d.offset + col_off,
        [[pstep, 128], [t_step, n_t], [1, inner]],
    )


def _build_program():
    nc = bacc.Bacc("TRN2", target_bir_lowering=False, debug=False)

    band_t = nc.dram_tensor("band_t", [128, NCB, RW], BF16, kind="ExternalInput").ap()
    xt_t = nc.dram_tensor("xt_t", [128, NCB, B], BF16, kind="ExternalInput").ap()
    al32 = nc.dram_tensor("al32", [N], F32, kind="ExternalInput").ap()
    strip_t = nc.dram_tensor("strip_t", [128, WCOLS], BF16, kind="ExternalInput").ap()
    out = nc.dram_tensor("out", [B, RW], BF16, kind="ExternalOutput").ap()

    with tile.TileContext(nc) as tc:
        with (
            tc.tile_pool(name="small", bufs=1) as sp,
            tc.tile_pool(name="agp", bufs=1) as agp,
            tc.tile_pool(name="bsb", bufs=1) as bsbp,
            tc.tile_pool(name="wt", bufs=8) as wtp,
            tc.tile_pool(name="xtp", bufs=1) as xtp,
            tc.tile_pool(name="opool", bufs=2) as op,
            tc.tile_pool(name="psum", bufs=1, space="PSUM") as pp,
            tc.tile_pool(name="psum_s", bufs=1, space="PSUM") as pps,
        ):
            # ---- all input DMAs issued first, consumption order per ring ----
            agf = agp.tile([128, WCOLS], BF16)
            bsb = bsbp.tile([128, NCB, RW], BF16)
            xt = xtp.tile([128, NCB, B], BF16)
            alpha_sb = sp.tile([128, N // 128], F32)

            def issue(eng, kind, i):
                if kind == "w":
                    c0, c1 = WCHUNKS[i]
                    eng.dma_start(agf[:, c0:c1], strip_t[:, c0:c1])
                elif kind == "b":
                    k0, k1 = BCHUNKS[i]
                    eng.dma_start(bsb[:, k0:k1, :], band_t[:, k0:k1, :])
                elif kind == "x":
                    k0, k1 = XCHUNKS[i]
                    eng.dma_start(xt[:, k0:k1, :], xt_t[:, k0:k1, :])
                else:
                    eng.dma_start(
                        alpha_sb[:], al32.rearrange("(p f) -> p f", p=128)
                    )

            for kind, i in SYNC_ORDER:
                issue(nc.sync, kind, i)
            for kind, i in SCAL_ORDER:
                issue(nc.scalar, kind, i)
            for kind, i in GP_ORDER:
                issue(nc.gpsimd, kind, i)

            # ---- normalizer: invK = sum(exp(alpha))/K, cK = K/sum ----
            exp_sb = sp.tile([128, N // 128], F32)
            rowsum = sp.tile([128, 1], F32)
            # alpha is uniform in [0,1): no max-subtraction needed for stability
            nc.scalar.activation(
                exp_sb[:], alpha_sb[:], mybir.ActivationFunctionType.Exp,
                accum_out=rowsum[:],
            )
            ones = sp.tile([128, 128], F32)
            nc.vector.memset(ones[:], 1.0)
            tot_ps = pps.tile([128, 1], F32)
            # total = ones.T @ rowsum -> per-partition copy of the full sum
            nc.tensor.matmul(tot_ps[:], ones[:], rowsum[:], start=True, stop=True)
            invK = sp.tile([128, 1], F32)
            nc.vector.tensor_scalar_mul(invK[:], tot_ps[:], 1.0 / float(K_TOPK))
            inv_s = sp.tile([128, 1], F32)
            nc.vector.reciprocal(inv_s[:], tot_ps[:])
            cK = sp.tile([128, 1], F32)
            nc.vector.tensor_scalar_mul(cK[:], inv_s[:], float(K_TOPK))

            # ---- strip: slice-wise exp (ACT) then min(strip, invK) (DVE;
            # contiguous tensor_scalar at ~350ns vs 745ns for a fused
            # scalar_tensor_tensor, so separate passes win).  The two late
            # xT chunk issues ride between early slices. ----
            n_slices = (WCOLS + SLICE - 1) // SLICE
            for s in range(n_slices):
                cols = slice(s * SLICE, min((s + 1) * SLICE, WCOLS))
                nc.scalar.activation(
                    agf[:, cols], agf[:, cols],
                    mybir.ActivationFunctionType.Exp,
                )
                nc.vector.tensor_scalar_min(agf[:, cols], agf[:, cols],
                                            invK[:, 0:1])

            # ---- main loop: per-block wt = band_k * strip_k (2D-contiguous
            # operands keep DVE in its fast perf mode) -> matmul pair ----
            psum0 = pp.tile([128, RW], F32)
            psum1 = pp.tile([128, RW], F32)
            for k in range(NCB):
                wt = wtp.tile([128, RW], BF16, tag=f"wt{k % 8}")
                nc.vector.tensor_tensor(
                    wt[:], bsb[:, k, :], agf[:, k * CB:k * CB + RW],
                    mybir.AluOpType.mult,
                )
                nc.tensor.matmul(psum0[:], xt[:, k, 0:128], wt[:],
                                 start=(k == 0), stop=(k == NCB - 1))
                nc.tensor.matmul(psum1[:], xt[:, k, 128:256], wt[:],
                                 start=(k == 0), stop=(k == NCB - 1))

            # ---- PSUM -> (x cK) -> SBUF bf16 -> DRAM, one store per ring ----
            o0 = op.tile([128, RW], BF16)
            nc.vector.tensor_scalar_mul(o0[:], psum0[:], cK[:, 0:1])
            nc.sync.dma_start(out[0:128, :], o0[:])
            o1 = op.tile([128, RW], BF16)
            nc.scalar.activation(o1[:], psum1[:],
                                 mybir.ActivationFunctionType.Copy,
                                 bias=0.0, scale=cK[:, 0:1])
            nc.scalar.dma_start(out[128:256, :], o1[:])

    nc.compile()
    return nc


_NC_CACHE = []


def _get_program():
    if not _NC_CACHE:
        _NC_CACHE.append(_build_program())
    return _NC_CACHE[0]


def prepare_in_maps(x: np.ndarray, V: np.ndarray, alpha: np.ndarray):
    """Shard + lay out the full inputs into 8 per-core input maps (bf16)."""
    x = np.ascontiguousarray(np.asarray(x, dtype=np.float32))
    V = np.ascontiguousarray(np.asarray(V, dtype=np.float32))
    alpha = np.ascontiguousarray(np.asarray(alpha, dtype=np.float32))

    # rows presented in reversed order (c = N-1-p); see module docstring
    xT_rev = np.ascontiguousarray(x.T[::-1, :]).astype(NP_BF16)  # (N, B)
    xt_t = np.ascontiguousarray(
        xT_rev.reshape(NCB, 128, B).transpose(1, 0, 2)
    )  # [p, k, b]

    # VtD[c, t] = V[t % N, c] for t in [0, 2N): doubled transpose for wrap-free
    # band extraction. band_m[c, j] = V[(r0 + j - c) % N, c] = VtD[c, N+r0+j-c]
    Vt = np.ascontiguousarray(V.T).astype(NP_BF16)
    VtD = np.concatenate([Vt, Vt], axis=1)  # (N, 2N) bf16
    flat = VtD.reshape(-1)
    isz = flat.itemsize

    in_maps = []
    for m in range(NCORES):
        r0 = m * RW
        start = N + r0  # element offset of band_m[0, 0] in flat
        band_m = np.lib.stride_tricks.as_strided(
            flat[start:], shape=(N, RW), strides=((2 * N - 1) * isz, isz),
        )
        band_rev = np.ascontiguousarray(band_m[::-1, :])  # (N, RW) bf16
        band_t = np.ascontiguousarray(
            band_rev.reshape(NCB, 128, RW).transpose(1, 0, 2)
        )  # [p, k, j]
        am = np.roll(alpha, -r0)
        a2 = np.concatenate([am, am]).astype(NP_BF16)
        # strip_t[p, u] = a2[1 + p + u]  (raw values; exp/min run on device)
        strip = np.ascontiguousarray(np.lib.stride_tricks.as_strided(
            a2[1:], shape=(128, WCOLS), strides=(a2.itemsize, a2.itemsize),
        ))
        in_maps.append({
            "band_t": band_t,
            "xt_t": xt_t,
            "al32": alpha,
            "strip_t": strip,
        })
    return in_maps


def gather_output(results) -> np.ndarray:
    return np.concatenate(
        [results[m]["out"].astype(np.float32) for m in range(NCORES)], axis=1
    )


def kernel(x: np.ndarray, V: np.ndarray, alpha: np.ndarray) -> np.ndarray:
    in_maps = prepare_in_maps(x, V, alpha)
    nc = _get_program()
    res = run_bass_kernel_spmd(nc, in_maps, core_ids=list(range(NCORES)))
    return gather_output(res.results)


# revision 34
# speedup vs baseline: 1.0588x; 1.0229x over previous
"""Trainium2 Bass kernel for nn_CustomFullyConnectedLayerGoogleTopK.

Reference computation:
    a = clip(K * softmax(alpha), 0, 1)                    # (4096,)
    W[rows, cols] += (V * a[:, None])  with rows=(j+i)%N, cols=j
    out = x @ W.T                                          # (256, 4096)

The scatter indices form a bijection (for each col j, row (j+i)%N hits every
row exactly once as i varies), so there is no actual accumulation:

    W[r, c] = V[(r - c) % N, c] * a[(r - c) % N]
    out[b, r] = sum_c x[b, c] * V[(r-c)%N, c] * a[(r-c)%N]

Sharding: output columns r are sharded 8 ways (512 per core) -> no collective;
each core reads only the diagonal band of V it needs, all of x, and produces a
disjoint out[:, r0:r0+512] slice.

The kernel is memory-bound, so the band and xT inputs are fed to the device as
bfloat16 (host-side cast; measured end-to-end max rel err ~3e-3 vs the fp32
reference, inside the 2e-2 gate), halving HBM traffic to ~7.6 MB per core.
PSUM accumulation stays fp32 across the full 4096-deep contraction.

Device-side layout trick: with the contraction rows presented in REVERSED
order (c = N-1-p for SBUF partition-row p), the skewed scale field the band
tiles need becomes the ascending Toeplitz  scale[p, j] = a2[1 + p + j]  where
a2 is the rolled-by-r0, doubled raw alpha.  One compact strip
agf[128, 4480] = a2[1 + p + u]  serves every contraction block k via the
column-offset view agf[:, 128k : 128k+512].

Scale algebra:  clip(K*softmax(alpha), 0, 1) = cK * min(exp(alpha), invK)
with invK = sum(exp(alpha))/K and cK = K/sum(exp(alpha)).  So the pipeline is
  - ACT: in-place Exp over strip slices (no bias -> starts as soon as the
    strip DMA lands; no Ln table ever loads),
  - DVE: in-place min(strip, invK) as two big contiguous passes, then one
    tensor_tensor  wt_k = band_k * strip[:, 128k:128k+512]  PER BLOCK with
    fully contiguous 2D operands (a strided multi-block view of the strip
    knocks the DVE out of its fast perf mode: measured 2292 ns per 4-block
    batch vs ~contiguous per-block passes; GpSimd elementwise is 21x slower
    than DVE and also locks DVE out of the shared SBUF port -> never used),
  - the cK factor rides the PSUM evictions (DVE for psum0, ACT Copy-with-
    scale for psum1, in parallel).

The strip is materialized by the HOST in plain partition-major layout
(strip_t[p, u] = a2[1+p+u], a pure replication of the raw alpha values) and
DMA'd as an ordinary contiguous 1.12 MB read: reading it amplified from the
tiny 16 KB alpha region on-device measured only ~85 GB/s on both the SWDGE
and HWDGE paths (the whole read concentrates on 1-2 HBM pages), which put it
on the critical path.

DMA layout: Sync carries the 2 strip chunks interleaved with the 9 band
chunks in consumption order plus one output store; Scalar carries the
compact alpha + 4 xT chunks up front and the other output store.  Band/xT
are pre-tiled on the host to partition-major [128, k, :] so every DMA
descriptor is a >=2 KB contiguous run.  The last two band chunks are 2
blocks instead of 4 to shorten the post-last-byte tail, and outputs are
stored as bf16 (upcast on host) to halve the tail store.
"""

import math
import os
import sys

import numpy as np

for _p in ("/opt/trn_rl_repo", "/root/.axon_site/_ro/trn_rl_repo"):
    if os.path.isdir(_p) and _p not in sys.path:
        sys.path.append(_p)

import ml_dtypes

import concourse.bacc as bacc
import concourse.bass as bass
import concourse.mybir as mybir
import concourse.tile as tile
from concourse.bass_utils import run_bass_kernel_spmd

F32 = mybir.dt.float32
BF16 = mybir.dt.bfloat16
NP_BF16 = ml_dtypes.bfloat16

N = 4096          # IN_F == OUT_F == N_PERM == DIAG
B = 256           # batch
NCORES = 8
RW = N // NCORES  # 512 output columns per core
K_TOPK = 3687     # ceil(0.9 * 4096 * 4096 / 4096)
CB = 128          # contraction block (SBUF partition count)
NCB = N // CB     # 32 contraction blocks
WCOLS = (NCB - 1) * CB + RW  # 4480: width of the Toeplitz scale strip

# band chunk boundaries (in blocks): small first chunks (fast DVE start),
# bigger middle (fewer DMAs -> fewer HWDGE-sem recycles), small tail
BCHUNKS = [(0, 4), (4, 8), (8, 14), (14, 20), (20, 26), (26, 30), (30, 32)]
# strip chunk boundaries (in strip columns): small first chunk
WCHUNKS = [(0, 1024), (1024, 2240), (2240, 4480)]
# xT chunk boundaries (in blocks)
XCHUNKS = [(0, 8), (8, 20), (20, 32)]
# Sync carries the first band chunk right behind the small first strip chunk
# (both gate the first wt multiply), then the rest of the strip, then the
# band in consumption order, then one output store.  Scalar (=ACT) carries
# ONLY DMAs that issue before its compute starts (alpha + xT) plus the last
# output store: a DMA issue stalled on a recycled HWDGE semaphore blocks
# everything behind it in that engine's FIFO.  (Moving xT to the SWDGE ring
# as a third DMA path was tried and hard-crashed the device —
# NRT_EXEC_UNIT_UNRECOVERABLE — do not revisit.)
SYNC_ORDER = [("w", 0), ("b", 0), ("w", 1), ("w", 2), ("b", 1), ("b", 2),
              ("b", 3), ("b", 4), ("b", 5), ("b", 6)]
SCAL_ORDER = [("a", 0), ("x", 0), ("x", 1), ("x", 2)]
GP_ORDER = []
SLICE = 512       # strip exp/min processing slice width (9 slices)


def _strided_cols(ap2d, col_off, t_step, n_t, inner):
    """[128, W] SBUF tile -> [128, n_t, inner] view starting at col_off with
    column stride t_step between t-slices (overlap allowed)."""
    pstep = ap2d.ap[0][0]
    return bass.AP(
        ap2d.tensor, ap2d.offset + col_off,
        [[pstep, 128], [t_step, n_t], [1, inner]],
    )


def _build_program():
    nc = bacc.Bacc("TRN2", target_bir_lowering=False, debug=False)

    band_t = nc.dram_tensor("band_t", [128, NCB, RW], BF16, kind="ExternalInput").ap()
    xt_t = nc.dram_tensor("xt_t", [128, NCB, B], BF16, kind="ExternalInput").ap()
    al32 = nc.dram_tensor("al32", [N], F32, kind="ExternalInput").ap()
    strip_t = nc.dram_tensor("strip_t", [128, WCOLS], BF16, kind="ExternalInput").ap()
    out = nc.dram_tensor("out", [B, RW], BF16, kind="ExternalOutput").ap()

    with tile.TileContext(nc) as tc:
        with (
            tc.tile_pool(name="small", bufs=1) as sp,
            tc.tile_pool(name="agp", bufs=1) as agp,
            tc.tile_pool(name="bsb", bufs=1) as bsbp,
            tc.tile_pool(name="wt", bufs=8) as wtp,
            tc.tile_pool(name="xtp", bufs=1) as xtp,
            tc.tile_pool(name="opool", bufs=2) as op,
            tc.tile_pool(name="psum", bufs=1, space="PSUM") as pp,
            tc.tile_pool(name="psum_s", bufs=1, space="PSUM") as pps,
        ):
            # ---- all input DMAs issued first, consumption order per ring ----
            agf = agp.tile([128, WCOLS], BF16)
            bsb = bsbp.tile([128, NCB, RW], BF16)
            xt = xtp.tile([128, NCB, B], BF16)
            alpha_sb = sp.tile([128, N // 128], F32)

            def issue(eng, kind, i):
                if kind == "w":
                    c0, c1 = WCHUNKS[i]
                    eng.dma_start(agf[:, c0:c1], strip_t[:, c0:c1])
                elif kind == "b":
                    k0, k1 = BCHUNKS[i]
                    eng.dma_start(bsb[:, k0:k1, :], band_t[:, k0:k1, :])
                elif kind == "x":
                    k0, k1 = XCHUNKS[i]
                    eng.dma_start(xt[:, k0:k1, :], xt_t[:, k0:k1, :])
                else:
                    eng.dma_start(
                        alpha_sb[:], al32.rearrange("(p f) -> p f", p=128)
                    )

            # high_priority hoists the issues ahead of the compiler-inserted
            # ACT_TABLE_LOAD on the Scalar queue (saves ~1.3us of alpha/xT
            # stream start in every measured trace)
            with tc.high_priority():
                for kind, i in SYNC_ORDER:
                    issue(nc.sync, kind, i)
                for kind, i in SCAL_ORDER:
                    issue(nc.scalar, kind, i)
                for kind, i in GP_ORDER:
                    issue(nc.gpsimd, kind, i)

            # ---- normalizer: invK = sum(exp(alpha))/K, cK = K/sum ----
            exp_sb = sp.tile([128, N // 128], F32)
            rowsum = sp.tile([128, 1], F32)
            # alpha is uniform in [0,1): no max-subtraction needed for stability
            nc.scalar.activation(
                exp_sb[:], alpha_sb[:], mybir.ActivationFunctionType.Exp,
                accum_out=rowsum[:],
            )
            ones = sp.tile([128, 128], F32)
            nc.vector.memset(ones[:], 1.0)
            tot_ps = pps.tile([128, 1], F32)
            # total = ones.T @ rowsum -> per-partition copy of the full sum
            nc.tensor.matmul(tot_ps[:], ones[:], rowsum[:], start=True, stop=True)
            invK = sp.tile([128, 1], F32)
            nc.vector.tensor_scalar_mul(invK[:], tot_ps[:], 1.0 / float(K_TOPK))
            inv_s = sp.tile([128, 1], F32)
            nc.vector.reciprocal(inv_s[:], tot_ps[:])
            cK = sp.tile([128, 1], F32)
            nc.vector.tensor_scalar_mul(cK[:], inv_s[:], float(K_TOPK))

            # ---- strip: slice-wise exp (ACT) then min(strip, invK) (DVE;
            # contiguous tensor_scalar at ~350ns vs 745ns for a fused
            # scalar_tensor_tensor, so separate passes win).  The two late
            # xT chunk issues ride between early slices. ----
            n_slices = (WCOLS + SLICE - 1) // SLICE
            for s in range(n_slices):
                cols = slice(s * SLICE, min((s + 1) * SLICE, WCOLS))
                nc.scalar.activation(
                    agf[:, cols], agf[:, cols],
                    mybir.ActivationFunctionType.Exp,
                )
                nc.vector.tensor_scalar_min(agf[:, cols], agf[:, cols],
                                            invK[:, 0:1])

            # ---- main loop: per-block wt = band_k * strip_k (2D-contiguous
            # operands keep DVE in its fast perf mode) -> matmul pair ----
            psum0 = pp.tile([128, RW], F32)
            psum1 = pp.tile([128, RW], F32)
            for k in range(NCB):
                wt = wtp.tile([128, RW], BF16, tag=f"wt{k % 8}")
                nc.vector.tensor_tensor(
                    wt[:], bsb[:, k, :], agf[:, k * CB:k * CB + RW],
                    mybir.AluOpType.mult,
                )
                nc.tensor.matmul(psum0[:], xt[:, k, 0:128], wt[:],
                                 start=(k == 0), stop=(k == NCB - 1))
                nc.tensor.matmul(psum1[:], xt[:, k, 128:256], wt[:],
                                 start=(k == 0), stop=(k == NCB - 1))

            # ---- PSUM -> (x cK) -> SBUF bf16 -> DRAM, one store per ring ----
            o0 = op.tile([128, RW], BF16)
            nc.vector.tensor_scalar_mul(o0[:], psum0[:], cK[:, 0:1])
            nc.sync.dma_start(out[0:128, :], o0[:])
            o1 = op.tile([128, RW], BF16)
            nc.scalar.activation(o1[:], psum1[:],
                                 mybir.ActivationFunctionType.Copy,
                                 bias=0.0, scale=cK[:, 0:1])
            nc.scalar.dma_start(out[128:256, :], o1[:])

    nc.compile()
    return nc


_NC_CACHE = []


def _get_program():
    if not _NC_CACHE:
        _NC_CACHE.append(_build_program())
    return _NC_CACHE[0]


def prepare_in_maps(x: np.ndarray, V: np.ndarray, alpha: np.ndarray):
    """Shard + lay out the full inputs into 8 per-core input maps (bf16)."""
    x = np.ascontiguousarray(np.asarray(x, dtype=np.float32))
    V = np.ascontiguousarray(np.asarray(V, dtype=np.float32))
    alpha = np.ascontiguousarray(np.asarray(alpha, dtype=np.float32))

    # rows presented in reversed order (c = N-1-p); see module docstring
    xT_rev = np.ascontiguousarray(x.T[::-1, :]).astype(NP_BF16)  # (N, B)
    xt_t = np.ascontiguousarray(
        xT_rev.reshape(NCB, 128, B).transpose(1, 0, 2)
    )  # [p, k, b]

    # VtD[c, t] = V[t % N, c] for t in [0, 2N): doubled transpose for wrap-free
    # band extraction. band_m[c, j] = V[(r0 + j - c) % N, c] = VtD[c, N+r0+j-c]
    Vt = np.ascontiguousarray(V.T).astype(NP_BF16)
    VtD = np.concatenate([Vt, Vt], axis=1)  # (N, 2N) bf16
    flat = VtD.reshape(-1)
    isz = flat.itemsize

    in_maps = []
    for m in range(NCORES):
        r0 = m * RW
        start = N + r0  # element offset of band_m[0, 0] in flat
        band_m = np.lib.stride_tricks.as_strided(
            flat[start:], shape=(N, RW), strides=((2 * N - 1) * isz, isz),
        )
        band_rev = np.ascontiguousarray(band_m[::-1, :])  # (N, RW) bf16
        band_t = np.ascontiguousarray(
            band_rev.reshape(NCB, 128, RW).transpose(1, 0, 2)
        )  # [p, k, j]
        am = np.roll(alpha, -r0)
        a2 = np.concatenate([am, am]).astype(NP_BF16)
        # strip_t[p, u] = a2[1 + p + u]  (raw values; exp/min run on device)
        strip = np.ascontiguousarray(np.lib.stride_tricks.as_strided(
            a2[1:], shape=(128, WCOLS), strides=(a2.itemsize, a2.itemsize),
        ))
        in_maps.append({
            "band_t": band_t,
            "xt_t": xt_t,
            "al32": alpha,
            "strip_t": strip,
        })
    return in_maps


def gather_output(results) -> np.ndarray:
    return np.concatenate(
        [results[m]["out"].astype(np.float32) for m in range(NCORES)], axis=1
    )


def kernel(x: np.ndarray, V: np.ndarray, alpha: np.ndarray) -> np.ndarray:
    in_maps = prepare_in_maps(x, V, alpha)
    nc = _get_program()
    res = run_bass_kernel_spmd(nc, in_maps, core_ids=list(range(NCORES)))
    return gather_output(res.results)
